# revision 1
# baseline (speedup 1.0000x reference)
"""Sparse L1-distance attention (nn_L1AttnSparse) on 8 Trainium2 NeuronCores.

Sharding: dst tokens are split across the 8 cores (256 dst tokens each);
every core keeps the full k/v tables (8 MB each) in DRAM and uses the
custom SWDGE gather instruction (dma_gather / InstDMAGatherAnt) to pull
the 2 KB k/v rows for its edges.  Scores, softmax over the 32 slots and
the weighted v-sum run on DVE/ACT.  Batch index is folded into the gather
index (tables are [2*2048, 512]).
"""

import sys

sys.path.insert(0, "/opt/trn_rl_repo")

import numpy as np

import concourse.bass as bass
import concourse.tile as tile
from concourse import bacc, mybir
from concourse.bass_utils import run_bass_kernel_spmd

BS = 2
N_TOK = 2048
NH = 8
W = 64
S = 32  # dst_mxlen
HW = NH * W  # 512 floats per (b, tok) row
N_CORES = 8
DT = N_TOK // N_CORES  # dst tokens per core = 256
CHUNKS = DT // 128  # dst chunks of 128 per core = 2
SH = 4  # slot halves per chunk (gather granularity)
SLOTS_PER = S // SH  # 16
IDX_PER = 128 * SLOTS_PER  # 2048 indices per gather


def _wrap_idx(flat):
    """int16 index list -> [128, n/16] tile layout: idx i at [i%16, i//16],
    replicated down the 8 groups of 16 partitions."""
    n = flat.shape[0]
    w16 = np.zeros((16, n // 16), dtype=np.int16)
    w16[np.arange(n) % 16, np.arange(n) // 16] = flat
    return np.tile(w16, (8, 1))


def build_kernel():
    nc = bacc.Bacc(
        "TRN2", target_bir_lowering=False, debug=False, num_devices=N_CORES,
        dynamic_dma_scratch_size=16384 * 8,
    )
    f32 = mybir.dt.float32
    i16 = mybir.dt.int16

    kf = nc.dram_tensor("kf", [BS * N_TOK, HW], f32, kind="ExternalInput").ap()
    vf = nc.dram_tensor("vf", [BS * N_TOK, HW], f32, kind="ExternalInput").ap()
    qc = nc.dram_tensor("qc", [BS, CHUNKS, 128, HW], f32, kind="ExternalInput").ap()
    idx = nc.dram_tensor(
        "idx", [BS, CHUNKS, SH, 128, IDX_PER // 16], i16, kind="ExternalInput"
    ).ap()
    oc = nc.dram_tensor("oc", [BS, CHUNKS, 128, HW], f32, kind="ExternalOutput").ap()

    with tile.TileContext(nc) as tc:
        with (
            tc.tile_pool(name="big", bufs=4) as bigp,
            tc.tile_pool(name="small", bufs=3) as smp,
            tc.tile_pool(name="idxp", bufs=4) as idxp,
        ):
            for b in range(BS):
                for c in range(CHUNKS):
                    q_t = smp.tile([128, HW], f32, tag="q")
                    nc.sync.dma_start(out=q_t[:], in_=qc[b, c])
                    L = smp.tile([128, S * NH], f32, tag="L")
                    idx_ts = []
                    for sh in range(SH):
                        it = idxp.tile([128, IDX_PER // 16], i16, tag=f"idx{sh}")
                        nc.sync.dma_start(out=it[:], in_=idx[b, c, sh])
                        idx_ts.append(it)
                    for sh in range(SH):
                        kg = bigp.tile([128, SLOTS_PER, HW], f32, tag="g")
                        nc.gpsimd.dma_gather(
                            kg[:], kf, idx_ts[sh][:], IDX_PER, IDX_PER, HW,
                            queue_num=0,
                        )
                        # kg <- kg - q (broadcast q over the slot dim)
                        nc.vector.tensor_tensor(
                            out=kg[:],
                            in0=kg[:],
                            in1=q_t[:, None, :].to_broadcast([128, SLOTS_PER, HW]),
                            op=mybir.AluOpType.subtract,
                        )
                        # L[:, sh half] = sum_w |kg|   ([128, s*h])
                        nc.vector.tensor_reduce(
                            out=L[:, sh * SLOTS_PER * NH : (sh + 1) * SLOTS_PER * NH],
                            in_=kg[:].rearrange("p s (h w) -> p (s h) w", w=W),
                            axis=mybir.AxisListType.X,
                            op=mybir.AluOpType.add,
                            apply_absolute_value=True,
                        )
                    # --- softmax over s (strided views: L is [p, (s h)]) ---
                    Lv = L[:].rearrange("p (s h) -> p h s", h=NH)
                    Lmin = smp.tile([128, NH], f32, tag="lmin")
                    nc.vector.tensor_reduce(
                        out=Lmin[:], in_=Lv, axis=mybir.AxisListType.X,
                        op=mybir.AluOpType.min,
                    )
                    E = smp.tile([128, S * NH], f32, tag="E")
                    nc.vector.tensor_tensor(
                        out=E[:].rearrange("p (s h) -> p s h", h=NH),
                        in0=L[:].rearrange("p (s h) -> p s h", h=NH),
                        in1=Lmin[:, None, :].to_broadcast([128, S, NH]),
                        op=mybir.AluOpType.subtract,
                    )
                    nc.scalar.activation(
                        out=E[:], in_=E[:], func=mybir.ActivationFunctionType.Exp,
                        scale=-1.0 / np.sqrt(W),
                    )
                    den = smp.tile([128, NH], f32, tag="den")
                    nc.vector.tensor_reduce(
                        out=den[:],
                        in_=E[:].rearrange("p (s h) -> p h s", h=NH),
                        axis=mybir.AxisListType.X,
                        op=mybir.AluOpType.add,
                    )
                    rden = smp.tile([128, NH], f32, tag="rden")
                    nc.vector.reciprocal(rden[:], den[:])
                    Wt = smp.tile([128, S * NH], f32, tag="Wt")
                    nc.vector.tensor_tensor(
                        out=Wt[:].rearrange("p (s h) -> p s h", h=NH),
                        in0=E[:].rearrange("p (s h) -> p s h", h=NH),
                        in1=rden[:, None, :].to_broadcast([128, S, NH]),
                        op=mybir.AluOpType.mult,
                    )
                    # --- weighted v gather+sum ---
                    ot = None
                    for sh in range(SH):
                        vg = bigp.tile([128, SLOTS_PER, HW], f32, tag="g")
                        nc.gpsimd.dma_gather(
                            vg[:], vf, idx_ts[sh][:], IDX_PER, IDX_PER, HW,
                            queue_num=0,
                        )
                        wslice = Wt[:, sh * SLOTS_PER * NH : (sh + 1) * SLOTS_PER * NH]
                        nc.vector.tensor_tensor(
                            out=vg[:].rearrange("p s (h w) -> p s h w", w=W),
                            in0=vg[:].rearrange("p s (h w) -> p s h w", w=W),
                            in1=wslice.rearrange("p (s h) -> p s h", h=NH)[
                                :, :, :, None
                            ].to_broadcast([128, SLOTS_PER, NH, W]),
                            op=mybir.AluOpType.mult,
                        )
                        on = smp.tile([128, HW], f32, tag="on")
                        nc.vector.tensor_reduce(
                            out=on[:],
                            in_=vg[:].rearrange("p s hw -> p hw s"),
                            axis=mybir.AxisListType.X,
                            op=mybir.AluOpType.add,
                        )
                        if ot is None:
                            ot = on
                        else:
                            acc = smp.tile([128, HW], f32, tag="acc")
                            nc.vector.tensor_tensor(
                                out=acc[:], in0=ot[:], in1=on[:],
                                op=mybir.AluOpType.add,
                            )
                            ot = acc
                    nc.sync.dma_start(out=oc[b, c], in_=ot[:])
    nc.compile()
    return nc


_NC_CACHE = None
_LAST_IN_MAPS = None


def kernel(v, q, k, coo, dst_mxlen):
    global _NC_CACHE
    assert int(dst_mxlen) == S
    v = np.asarray(v, dtype=np.float32)
    q = np.asarray(q, dtype=np.float32)
    k = np.asarray(k, dtype=np.float32)
    coo = np.asarray(coo)

    # src table: srct[t, s] = src index of edge (dst=t, slot=s)
    srct = np.zeros((N_TOK, S), dtype=np.int64)
    srct[coo[:, 0], coo[:, 2]] = coo[:, 1]

    kf = k.reshape(BS * N_TOK, HW)
    vf = v.reshape(BS * N_TOK, HW)

    if _NC_CACHE is None:
        _NC_CACHE = build_kernel()
    nc = _NC_CACHE

    in_maps = []
    for core in range(N_CORES):
        lo = core * DT
        qc = q[:, lo : lo + DT].reshape(BS, CHUNKS, 128, HW)
        idx = np.zeros((BS, CHUNKS, SH, 128, IDX_PER // 16), dtype=np.int16)
        for b in range(BS):
            for c in range(CHUNKS):
                for sh in range(SH):
                    # index i = s_local*128 + p  ->  row b*2048 + srct[...]
                    sl = np.arange(SLOTS_PER) + sh * SLOTS_PER
                    flat = (
                        b * N_TOK
                        + srct[lo + c * 128 : lo + (c + 1) * 128, sl].T
                    ).reshape(-1).astype(np.int16)  # [s_local, p] -> flat
                    idx[b, c, sh] = _wrap_idx(flat)
        in_maps.append(
            {"kf": kf, "vf": vf, "qc": np.ascontiguousarray(qc), "idx": idx}
        )

    global _LAST_IN_MAPS
    _LAST_IN_MAPS = in_maps
    res = run_bass_kernel_spmd(nc, in_maps, list(range(N_CORES)))
    out = np.empty((BS, N_TOK, NH, W), dtype=np.float32)
    for core in range(N_CORES):
        lo = core * DT
        out[:, lo : lo + DT] = res.results[core]["oc"].reshape(BS, DT, NH, W)
    return out



# revision 14
# speedup vs baseline: 1.5003x; 1.5003x over previous
"""Sparse L1-distance attention (nn_L1AttnSparse) on 8 Trainium2 NeuronCores.

v2: bf16 tables + engine-balanced score pipeline + PE slot-sum.

Sharding: dst tokens split across 8 cores (256 each); per chunk of 128 dst:
  - gather k rows bf16 in dst-layout [p=dst, s, h*w] (one 4096-idx SWDGE gather)
  - d = k - q (TT, slots split DVE/Pool), |d| on Act, tree-sum over w via
    TT-adds (bf16 levels then f32 tail) -> scores L f32 [p, s, h]
  - E = exp(-L/8) bf16 on Act; den = sum_s E (DVE); rden = 1/den
  - E rearranged to edge layout [p=(s,d'), g, h] via a DRAM roundtrip
  - gather v rows bf16 in edge-layout [p=(s,d'), g, w*h] (v table w-major)
  - wv = vg * E_edge (TT); slot-sum on PE: psum[4g:4g+4,:] = onehot.T @ wv[:,g,:]
  - out = psum * rden -> bf16 [p, w, h]; host casts/transposes back
"""

import sys

sys.path.insert(0, "/opt/trn_rl_repo")

import ml_dtypes
import numpy as np

import concourse.bass as bass
import concourse.tile as tile
from concourse import bacc, mybir
from concourse.bass_utils import run_bass_kernel_spmd

BF16 = ml_dtypes.bfloat16

BS = 2
N_TOK = 2048
NH = 8
W = 64
S = 32  # dst_mxlen
HW = NH * W  # 512
N_CORES = 8
DT = N_TOK // N_CORES  # dst tokens per core = 256
CH = DT // 128  # chunks of 128 dst per batch = 2
G = 128 // 4  # dst groups of 4 per chunk = 32
NIDX = 128 * S  # 4096 gather indices per chunk
NG = 4  # gathers per table per chunk (NIDX/NG indices each)
A_DVE = 23  # score-side slots computed on DVE; rest on Pool
SCALE = -1.0 / np.sqrt(W)  # -0.125


def _wrap_idx(flat):
    """int16 index list -> [128, n/16] tile layout: idx i at [i%16, i//16],
    replicated down the 8 groups of 16 partitions."""
    n = flat.shape[0]
    w16 = np.zeros((16, n // 16), dtype=np.int16)
    w16[np.arange(n) % 16, np.arange(n) // 16] = flat
    return np.tile(w16, (8, 1))


def _score_tree(veng, kgv, Fv, Lv):
    """Sum over w (64) of |d| for kgv [128, ss, NH, W] bf16 (in-place tree),
    f32 tail via Fv [128, ss, NH, 4], result into Lv [128, ss, NH, 1] f32."""
    add = mybir.AluOpType.add

    def tt(o, a, b):
        veng.tensor_tensor(out=o, in0=a, in1=b, op=add)

    for half in (32, 16, 8):
        tt(kgv[:, :, :, 0:half], kgv[:, :, :, 0:half], kgv[:, :, :, half : 2 * half])
    tt(Fv, kgv[:, :, :, 0:4], kgv[:, :, :, 4:8])
    tt(Fv[:, :, :, 0:2], Fv[:, :, :, 0:2], Fv[:, :, :, 2:4])
    tt(Lv, Fv[:, :, :, 0:1], Fv[:, :, :, 1:2])


def build_kernel():
    nc = bacc.Bacc(
        "TRN2", target_bir_lowering=False, debug=False, num_devices=N_CORES,
        dynamic_dma_scratch_size=16384 * 4,
    )
    f32 = mybir.dt.float32
    bf16 = mybir.dt.bfloat16
    i16 = mybir.dt.int16
    Alu = mybir.AluOpType

    kf = nc.dram_tensor("kf", [BS * N_TOK, HW], bf16, kind="ExternalInput").ap()
    vf = nc.dram_tensor("vf", [BS * N_TOK, HW], bf16, kind="ExternalInput").ap()
    qc = nc.dram_tensor("qc", [BS, CH, 128, HW], bf16, kind="ExternalInput").ap()
    ik = nc.dram_tensor("ik", [BS, CH, 128, NIDX // 16], i16, kind="ExternalInput").ap()
    iv = nc.dram_tensor("iv", [BS, CH, 128, NIDX // 16], i16, kind="ExternalInput").ap()
    oh = nc.dram_tensor("oh", [128, 8 * 32], bf16, kind="ExternalInput").ap()
    oc = nc.dram_tensor("oc", [BS, CH, 128, HW], bf16, kind="ExternalOutput").ap()
    # DRAM scratch for the dst->edge layout rearrange of E
    sc = nc.dram_tensor("sc", [BS, CH, 128, S * NH], bf16, kind="Internal").ap()

    with tile.TileContext(nc) as tc:
        with (
            tc.tile_pool(name="kp", bufs=2) as kp,
            tc.tile_pool(name="vp", bufs=2) as vp,
            tc.tile_pool(name="sm", bufs=2) as sm,
            tc.tile_pool(name="cst", bufs=1) as cst,
            tc.tile_pool(name="pp", bufs=2, space="PSUM") as pp,
        ):
            oh_t = cst.tile([128, 8 * 32], bf16, tag="oh")
            nc.sync.dma_start(out=oh_t[:], in_=oh)
            for b in range(BS):
                for c in range(CH):
                    q_t = sm.tile([128, HW], bf16, tag="q")
                    nc.sync.dma_start(out=q_t[:], in_=qc[b, c])
                    ikt = sm.tile([128, NIDX // 16], i16, tag="ik")
                    nc.sync.dma_start(out=ikt[:], in_=ik[b, c])
                    ivt = sm.tile([128, NIDX // 16], i16, tag="iv")
                    nc.sync.dma_start(out=ivt[:], in_=iv[b, c])

                    # ---- scores ----
                    kg = kp.tile([128, S, HW], bf16, tag="kg")
                    nsl = S // NG  # slots per gather
                    for gi in range(NG):
                        nc.gpsimd.dma_gather(
                            kg[:, gi * nsl : (gi + 1) * nsl],
                            kf,
                            ikt[:, gi * (NIDX // NG // 16) : (gi + 1) * (NIDX // NG // 16)],
                            NIDX // NG,
                            NIDX // NG,
                            HW,
                            queue_num=0,
                        )
                    kgv = kg[:].rearrange("p s (h w) -> p s h w", w=W)
                    F = sm.tile([128, S, NH, 4], f32, tag="F")
                    L = sm.tile([128, S, NH, 1], f32, tag="L")
                    for eng, s0, s1 in (
                        (nc.vector, 0, A_DVE),
                        (nc.gpsimd, A_DVE, S),
                    ):
                        ss = s1 - s0
                        eng.tensor_tensor(
                            out=kg[:, s0:s1],
                            in0=kg[:, s0:s1],
                            in1=q_t[:, None, :].to_broadcast([128, ss, HW]),
                            op=Alu.subtract,
                        )
                        nc.scalar.activation(
                            out=kg[:, s0:s1].rearrange("p s hw -> p (s hw)"),
                            in_=kg[:, s0:s1].rearrange("p s hw -> p (s hw)"),
                            func=mybir.ActivationFunctionType.Abs,
                        )
                        _score_tree(
                            eng, kgv[:, s0:s1], F[:, s0:s1], L[:, s0:s1]
                        )

                    # ---- softmax pieces (normalization deferred to the end) ----
                    E = sm.tile([128, S * NH], bf16, tag="E")
                    nc.scalar.activation(
                        out=E[:],
                        in_=L[:].rearrange("p s h one -> p (s h one)"),
                        func=mybir.ActivationFunctionType.Exp,
                        scale=float(SCALE),
                    )
                    den = sm.tile([128, NH], f32, tag="den")
                    nc.vector.tensor_reduce(
                        out=den[:],
                        in_=E[:].rearrange("p (s h) -> p h s", h=NH),
                        axis=mybir.AxisListType.X,
                        op=Alu.add,
                    )
                    rden = sm.tile([128, NH], f32, tag="rden")
                    nc.vector.reciprocal(rden[:], den[:])

                    # ---- E -> edge layout via DRAM roundtrip ----
                    nc.sync.dma_start(out=sc[b, c], in_=E[:])
                    # edge layout p2 = d'*32 + s: each dp fills a contiguous
                    # 32-partition block of Ee
                    Ee = sm.tile([128, G, NH], bf16, tag="Ee")
                    sc_r = sc[b, c].rearrange("(g dp) (s h) -> dp s g h", dp=4, h=NH)
                    for dp in range(4):
                        nc.sync.dma_start(
                            out=Ee[32 * dp : 32 * dp + 32], in_=sc_r[dp]
                        )

                    # ---- weighted v gather + PE slot-sum ----
                    vg = vp.tile([128, G, HW], bf16, tag="vg")
                    ngl = G // NG  # groups per gather
                    for gi in range(NG):
                        nc.gpsimd.dma_gather(
                            vg[:, gi * ngl : (gi + 1) * ngl],
                            vf,
                            ivt[:, gi * (NIDX // NG // 16) : (gi + 1) * (NIDX // NG // 16)],
                            NIDX // NG,
                            NIDX // NG,
                            HW,
                            queue_num=0,
                        )
                    vgv = vg[:].rearrange("p g (w h) -> p g w h", h=NH)
                    nc.vector.tensor_tensor(
                        out=vgv,
                        in0=vgv,
                        in1=Ee[:, :, None, :].to_broadcast([128, G, W, NH]),
                        op=Alu.mult,
                    )
                    # 8 matmuls accumulate a 32-dst block: lhsT column
                    # (g%8)*4+d' selects dst 4g+d'; psA = dst 0..63, psB = 64..127
                    psA = pp.tile([64, HW], f32, tag="psA")
                    psB = pp.tile([64, HW], f32, tag="psB")
                    for g in range(G):
                        ps = psA if g < 16 else psB
                        off = 32 * ((g // 8) % 2)
                        nc.tensor.matmul(
                            out=ps[off : off + 32, :],
                            lhsT=oh_t[:, 32 * (g % 8) : 32 * (g % 8) + 32],
                            rhs=vg[:, g, :],
                            start=(g % 8 == 0),
                            stop=(g % 8 == 7),
                        )
                    out_t = sm.tile([128, HW], bf16, tag="out")
                    for half, ps in ((0, psA), (1, psB)):
                        nc.vector.tensor_tensor(
                            out=out_t[64 * half : 64 * half + 64].rearrange(
                                "p (w h) -> p w h", h=NH
                            ),
                            in0=ps[:].rearrange("p (w h) -> p w h", h=NH),
                            in1=rden[64 * half : 64 * half + 64, None, :].to_broadcast(
                                [64, W, NH]
                            ),
                            op=Alu.mult,
                        )
                    nc.sync.dma_start(out=oc[b, c], in_=out_t[:])
    nc.compile()
    return nc


_NC_CACHE = None


def kernel(v, q, k, coo, dst_mxlen):
    global _NC_CACHE
    assert int(dst_mxlen) == S
    v = np.asarray(v, dtype=np.float32)
    q = np.asarray(q, dtype=np.float32)
    k = np.asarray(k, dtype=np.float32)
    coo = np.asarray(coo)

    # src table: srct[t, s] = src index of edge (dst=t, slot=s)
    srct = np.zeros((N_TOK, S), dtype=np.int64)
    srct[coo[:, 0], coo[:, 2]] = coo[:, 1]

    kf = np.ascontiguousarray(k.reshape(BS * N_TOK, HW)).astype(BF16)
    # v table rows stored w-major ([w, h] per row)
    vf = np.ascontiguousarray(
        v.transpose(0, 1, 3, 2).reshape(BS * N_TOK, HW)
    ).astype(BF16)
    # one-hot lhsT blocks for edge partitions p2 = dp*32 + s:
    # ohm[p2, go, go*4 + dp] = 1
    ohm = np.zeros((128, 8, 32), dtype=BF16)
    p2 = np.arange(128)
    for go in range(8):
        ohm[p2, go, go * 4 + p2 // 32] = 1.0
    ohm = ohm.reshape(128, 256)

    if _NC_CACHE is None:
        _NC_CACHE = build_kernel()
    nc = _NC_CACHE

    in_maps = []
    for core in range(N_CORES):
        lo = core * DT
        qcc = q[:, lo : lo + DT].reshape(BS, CH, 128, HW).astype(BF16)
        ikm = np.zeros((BS, CH, 128, NIDX // 16), dtype=np.int16)
        ivm = np.zeros((BS, CH, 128, NIDX // 16), dtype=np.int16)
        for b in range(BS):
            for c in range(CH):
                sl = srct[lo + c * 128 : lo + (c + 1) * 128]  # [128 dst, 32 s]
                # k-gather: idx[s*128 + p] = row of (dst=p, slot=s)
                fk = (b * N_TOK + sl.T).reshape(-1).astype(np.int16)
                # v-gather: idx[g*128 + dp*32 + s] = row of (dst=4g+dp, slot=s)
                fv = (b * N_TOK + sl.reshape(G, 4, S)).reshape(-1).astype(np.int16)
                ikm[b, c] = _wrap_idx(fk)
                ivm[b, c] = _wrap_idx(fv)
        in_maps.append(
            {
                "kf": kf,
                "vf": vf,
                "qc": np.ascontiguousarray(qcc),
                "ik": ikm,
                "iv": ivm,
                "oh": ohm,
            }
        )

    res = run_bass_kernel_spmd(nc, in_maps, list(range(N_CORES)))
    out = np.empty((BS, N_TOK, NH, W), dtype=np.float32)
    for core in range(N_CORES):
        lo = core * DT
        r = np.asarray(res.results[core]["oc"]).astype(np.float32)
        # [BS, CH, 128, (w h)] -> [BS, DT, NH, W]
        out[:, lo : lo + DT] = r.reshape(BS, DT, W, NH).transpose(0, 1, 3, 2)
    return out
